# revision 2
# baseline (speedup 1.0000x reference)
"""Chamfer-distance loss kernel for Trainium2 (8 NeuronCores, SPMD).

Problem: loss = chamfer(coarse, gt_pts) + alpha * chamfer(fine, gt_pts)
  coarse [8,1024,3], fine [8,8192,3], gt [8,3,8192] (channel-first), alpha scalar.
  chamfer(x,y) = mean_n min_m d(n,m) + mean_m min_n d(n,m), d = squared L2.

Sharding: data-parallel over batch - one batch element per NeuronCore.

Per-core pipeline (v2 - DMA-transposed column path):
  - d is produced 128x512 at a time by the PE as a K=7 fp16 matmul holding the
    COMPLETE distance:
      lhsT rows {x0,x1,x2, x2hi,x2lo, 1,1}
      rhs  rows {-2y0,-2y1,-2y2, 1,1, y2hi,y2lo}
    (|x|^2 and |y|^2 enter at ~fp32 precision via fp16 hi/lo splits of the
    presummed squared norms), so PSUM = d in fp32.
  - Cast pass psum fp32 -> SBUF fp16 S: pure copy, split between ScalarE
    (ACTF.Copy) and VectorE (tensor_copy) ~5:1 - both engines are the only
    ones with PSUM access and are jointly the drain bottleneck.
  - Row direction (min over m): one tensor_scalar per tile at fp16 4x mode
    (op0=min vs large const = identity, op1=min into accum_out). Since psum
    holds full d, no bias post-add is needed.
  - Col direction (min over n): for most fine tiles, S is DMA-TRANSPOSED
    (XBAR 16-bit transpose, idle DMA engines) into a grouped buffer
    TG[m_p, j, i_local, n] for groups of GT=4 tiles; then per (group, j) one
    tensor_scalar 4x accum-min over free=[GT,128] yields uncontaminated
    per-m mins at ~0.32 ns/elem instead of the 0.52 ns/elem tensor_tensor
    running-min. Remaining fine tiles + all coarse tiles use the fp16 2x
    tensor_tensor running min (acc) with the PE-transpose partition collapse
    at the end (as v1).
  - (HW-verified: gpsimd/Pool tensor ops and DMA accum-min are ISA-rejected
    on TRN2; matmul psum out must be fp32; DVE 4x requires all-SBUF operands
    - which is what the transposed-S path arranges.)

Host does only O(N) prep and the final scalar arithmetic.
"""

import sys

sys.path.insert(0, "/opt/trn_rl_repo")

import numpy as np

B = 8
NF = 8192  # fine points
NC_ = 1024  # coarse points
M = 8192  # gt points

# --- tuning knobs ----------------------------------------------------------
GT = 4  # X-route group size (tiles per transposed group)
N_TT_FINE = 4  # fine tiles routed through the tensor_tensor path
CAST_DVE_MOD = 6  # every CAST_DVE_MOD-th cast group goes to VectorE

# --- module-level program cache -------------------------------------------
_PROGRAM = None
PROFILE = False
LAST_RESULTS = None


def _build_program():
    from concourse import bacc, bass, tile
    import concourse.mybir as mybir

    f16, f32 = mybir.dt.float16, mybir.dt.float32
    AL = mybir.AluOpType
    ACTF = mybir.ActivationFunctionType

    nc = bacc.Bacc("TRN2", target_bir_lowering=False, debug=False, num_devices=B)

    xaug_f = nc.dram_tensor("xaug_f", [7, NF], f16, kind="ExternalInput")
    xaug_c = nc.dram_tensor("xaug_c", [7, NC_], f16, kind="ExternalInput")
    yaug_d = nc.dram_tensor("yaug", [7, M], f16, kind="ExternalInput")
    iden_d = nc.dram_tensor("iden", [128, 128], f16, kind="ExternalInput")
    ones_d = nc.dram_tensor("ones128", [128, 1], f32, kind="ExternalInput")
    out_d = nc.dram_tensor("out", [1, 8], f32, kind="ExternalOutput")

    n_fine_tiles = NF // 128  # 64
    n_coarse_tiles = NC_ // 128  # 8
    tt_fine = set(range(n_fine_tiles - N_TT_FINE, n_fine_tiles))
    x_fine = [i for i in range(n_fine_tiles) if i not in tt_fine]
    assert len(x_fine) % GT == 0

    cast_counter = [0]

    with tile.TileContext(nc) as tc:
        with (
            tc.tile_pool(name="const", bufs=1) as cpool,
            tc.tile_pool(name="s", bufs=3) as spool,
            tc.tile_pool(name="scr", bufs=1) as scrpool,
            tc.tile_pool(name="tg", bufs=1) as tgpool,
            tc.tile_pool(name="ct", bufs=2) as ctpool,
            tc.tile_pool(name="fin", bufs=1) as fpool,
            tc.tile_pool(name="ps", bufs=2, space=bass.MemorySpace.PSUM) as pspool,
        ):
            Xf = cpool.tile([7, NF], f16)
            nc.sync.dma_start(Xf[:], xaug_f.ap())
            Xc = cpool.tile([7, NC_], f16)
            nc.sync.dma_start(Xc[:], xaug_c.ap())
            Y = cpool.tile([7, M], f16)
            nc.sync.dma_start(Y[:], yaug_d.ap())
            iden = cpool.tile([128, 128], f16)
            nc.sync.dma_start(iden[:], iden_d.ap())
            ones = cpool.tile([128, 1], f32)
            nc.sync.dma_start(ones[:], ones_d.ap())

            outb = cpool.tile([1, 8], f32)

            accf = cpool.tile([128, M], f16)  # tt-route fine col accumulator
            accc = cpool.tile([128, M], f16)  # coarse col accumulator
            rowWf = cpool.tile([128, n_fine_tiles], f32)
            rowWc = cpool.tile([128, n_coarse_tiles], f32)
            colWX = cpool.tile([128, M // 128], f32)  # X-route per-m col mins
            TG = tgpool.tile([128, M // 128, GT, 128], f16)

            def make_S(Xa, i):
                """matmuls + cast -> S fp16 [128, M] (complete d)."""
                S = spool.tile([128, M], f16, tag="S")
                for g in range(4):
                    ps = pspool.tile([128, 2048], f32, tag="ps")
                    for j in range(4):
                        mlo = g * 2048 + j * 512
                        nc.tensor.matmul(
                            ps[:, j * 512 : (j + 1) * 512],
                            lhsT=Xa[:, i * 128 : (i + 1) * 128],
                            rhs=Y[:, mlo : mlo + 512],
                            start=True,
                            stop=True,
                        )
                    c = cast_counter[0]
                    cast_counter[0] += 1
                    if c % CAST_DVE_MOD == CAST_DVE_MOD - 1:
                        nc.vector.tensor_copy(S[:, g * 2048 : (g + 1) * 2048], ps[:])
                    else:
                        nc.scalar.activation(
                            S[:, g * 2048 : (g + 1) * 2048],
                            ps[:],
                            ACTF.Copy,
                            bias=0.0,
                            scale=1.0,
                        )
                return S

            def row_min(S, rowW, i):
                scr = scrpool.tile([128, M], f16, tag="scr")
                nc.vector.tensor_scalar(
                    out=scr[:],
                    in0=S[:],
                    scalar1=60000.0,
                    scalar2=None,
                    op0=AL.min,
                    op1=AL.min,
                    accum_out=rowW[:, i : i + 1],
                )

            # ---- fine family: X-route groups + tt tail ----
            first_group = [True]

            def flush_group(nS):
                """col-reduce the TG buffer holding nS transposed tiles."""
                if first_group[0] and nS == GT:
                    tgt = colWX
                else:
                    tgt = ctpool.tile([128, M // 128], f32, tag="colT")
                for j in range(M // 128):
                    scr2 = scrpool.tile([128, GT * 128], f16, tag="scrj")
                    nc.vector.tensor_scalar(
                        out=scr2[:, 0 : nS * 128],
                        in0=TG[:, j, 0:nS, :],
                        scalar1=60000.0,
                        scalar2=None,
                        op0=AL.min,
                        op1=AL.min,
                        accum_out=tgt[:, j : j + 1],
                    )
                if tgt is not colWX:
                    nc.vector.tensor_tensor(
                        out=colWX[:], in0=colWX[:], in1=tgt[:], op=AL.min
                    )
                first_group[0] = False

            in_group = 0
            for i in range(n_fine_tiles):
                S = make_S(Xf, i)
                row_min(S, rowWf, i)
                if i in tt_fine:
                    if i == min(tt_fine):
                        nc.vector.tensor_copy(accf[:], S[:])
                    else:
                        nc.vector.tensor_tensor(
                            out=accf[:], in0=accf[:], in1=S[:], op=AL.min
                        )
                else:
                    nc.sync.dma_start(
                        TG[:, :, in_group, :], S[:], transpose=True
                    )
                    in_group += 1
                    if in_group == GT:
                        flush_group(GT)
                        in_group = 0
            assert in_group == 0

            # ---- coarse family: tt route ----
            for i in range(n_coarse_tiles):
                S = make_S(Xc, i)
                row_min(S, rowWc, i)
                if i == 0:
                    nc.vector.tensor_copy(accc[:], S[:])
                else:
                    nc.vector.tensor_tensor(
                        out=accc[:], in0=accc[:], in1=S[:], op=AL.min
                    )

            # ---- finals ----
            def row_total(rowW, oidx):
                rsum = fpool.tile([128, 1], f32, tag=f"rsum{oidx}")
                nc.vector.tensor_reduce(
                    out=rsum[:], in_=rowW[:], axis=mybir.AxisListType.X, op=AL.add
                )
                pr = pspool.tile([1, 1], f32, tag="ps")
                nc.tensor.matmul(pr[:], lhsT=rsum[:], rhs=ones[:], start=True, stop=True)
                nc.vector.tensor_copy(outb[0:1, oidx : oidx + 1], pr[:])

            def p_collapse(acc, cmb):
                """partition-collapse acc [128, M] f16 -> cmb [128, M//128] f32."""
                for c0 in range(0, M // 128, 4):
                    pst = pspool.tile([128, 4, 128], f16, tag="ps")
                    for q in range(4):
                        nc.tensor.transpose(
                            pst[:, q, :],
                            acc[:, (c0 + q) * 128 : (c0 + q + 1) * 128],
                            iden[:],
                        )
                    nc.vector.tensor_reduce(
                        out=cmb[:, c0 : c0 + 4],
                        in_=pst[:],
                        axis=mybir.AxisListType.X,
                        op=AL.min,
                    )

            def col_total(cmb, oidx):
                csum = fpool.tile([128, 1], f32, tag=f"csum{oidx}")
                nc.vector.tensor_reduce(
                    out=csum[:], in_=cmb[:], axis=mybir.AxisListType.X, op=AL.add
                )
                pc = pspool.tile([1, 1], f32, tag="ps")
                nc.tensor.matmul(pc[:], lhsT=csum[:], rhs=ones[:], start=True, stop=True)
                nc.vector.tensor_copy(outb[0:1, oidx : oidx + 1], pc[:])

            row_total(rowWf, 0)
            row_total(rowWc, 2)

            # fine col: merge tt-route collapse into colWX
            cmbf = fpool.tile([128, M // 128], f32, tag="cmbf")
            p_collapse(accf, cmbf)
            nc.vector.tensor_tensor(out=colWX[:], in0=colWX[:], in1=cmbf[:], op=AL.min)
            col_total(colWX, 1)

            cmbc = fpool.tile([128, M // 128], f32, tag="cmbc")
            p_collapse(accc, cmbc)
            col_total(cmbc, 3)

            nc.vector.memset(outb[0:1, 4:8], 0.0)
            nc.sync.dma_start(out_d.ap(), outb[:])

    nc.compile()
    return nc


def _get_program():
    global _PROGRAM
    if _PROGRAM is None:
        _PROGRAM = _build_program()
    return _PROGRAM


def _aug_x(x_b, n):
    """xaug [7, n]: rows x0,x1,x2, x2hi, x2lo, 1, 1 (fp16)."""
    f16 = np.float16
    xa = np.ones((7, n), f16)
    x16 = x_b.astype(f16)
    xa[0:3] = x16.T
    x2 = (x16.astype(np.float32) ** 2).sum(1)
    hi = x2.astype(f16)
    xa[3] = hi
    xa[4] = (x2 - hi.astype(np.float32)).astype(f16)
    return xa


def _prep_core_inputs(fine_b, coarse_b, gt_b):
    f16 = np.float16
    g16 = gt_b.astype(f16)  # [3, M]
    yaug = np.ones((7, M), f16)
    yaug[0:3] = (-2.0 * g16.astype(np.float32)).astype(f16)
    y2 = (g16.astype(np.float32) ** 2).sum(0)
    hi = y2.astype(f16)
    yaug[5] = hi
    yaug[6] = (y2 - hi.astype(np.float32)).astype(f16)
    return {
        "xaug_f": _aug_x(fine_b, NF),
        "xaug_c": _aug_x(coarse_b, NC_),
        "yaug": yaug,
        "iden": np.eye(128, dtype=f16),
        "ones128": np.ones((128, 1), np.float32),
    }


def kernel(coarse, fine, gt, alpha):
    global LAST_RESULTS
    from concourse import bass_utils

    coarse = np.asarray(coarse, np.float32)
    fine = np.asarray(fine, np.float32)
    gt = np.asarray(gt, np.float32)
    alpha = np.float32(np.asarray(alpha))

    nc = _get_program()
    in_maps = [_prep_core_inputs(fine[b], coarse[b], gt[b]) for b in range(B)]
    res = bass_utils.run_bass_kernel_spmd(
        nc, in_maps, core_ids=list(range(B)), trace=PROFILE
    )
    LAST_RESULTS = res
    per = np.stack([r["out"][0] for r in res.results]).astype(np.float64)  # [B, 8]
    lf = np.float32((per[:, 0] / NF + per[:, 1] / M).mean())
    lc = np.float32((per[:, 2] / NC_ + per[:, 3] / M).mean())
    loss = np.float32(lc + np.float32(alpha) * lf)
    return (loss, lc, lf)


if __name__ == "__main__":
    rng = np.random.default_rng(0)
    out = kernel(
        coarse=rng.standard_normal((B, NC_, 3)).astype(np.float32),
        fine=rng.standard_normal((B, NF, 3)).astype(np.float32),
        gt=rng.standard_normal((B, 3, M)).astype(np.float32),
        alpha=np.float32(1.0),
    )
    print(out)


# revision 3
# speedup vs baseline: 1.6124x; 1.6124x over previous
"""Chamfer-distance loss kernel for Trainium2 (8 NeuronCores, SPMD).

Problem: loss = chamfer(coarse, gt_pts) + alpha * chamfer(fine, gt_pts)
  coarse [8,1024,3], fine [8,8192,3], gt [8,3,8192] (channel-first), alpha scalar.
  chamfer(x,y) = mean_n min_m d(n,m) + mean_m min_n d(n,m), d = squared L2.

Sharding: data-parallel over batch - one batch element per NeuronCore.

Per-core pipeline (v3 - half-m DMA-transposed column path):
  - d is produced 128x512 at a time by the PE as a K=7 fp16 matmul holding the
    COMPLETE distance:
      lhsT rows {x0,x1,x2, x2hi,x2lo, 1,1}
      rhs  rows {-2y0,-2y1,-2y2, 1,1, y2hi,y2lo}
    (|x|^2, |y|^2 enter at ~fp32 precision via fp16 hi/lo splits of presummed
    norms), so PSUM = d in fp32.
  - Every (tile, m-half) produces S fp16 [128, 4096] via a cast pass split
    between ScalarE (ACTF.Copy) and VectorE (tensor_copy) - these two are the
    only engines with PSUM access and jointly form the drain bottleneck.
  - Row direction: one tensor_scalar per (tile, half) at fp16 4x mode
    (op0=min vs large const, op1=min into accum_out rowW[:, i, mh]).
  - Col direction:
      * fine m-lo half: S is DMA-TRANSPOSED (XBAR, idle DMA engines) into
        TG[m_p, j, slot, n] for groups of GT=4 tiles (TG 32KB, double-
        buffered); per (group, j) one tensor_scalar 4x accum-min over
        free=[4,128] yields uncontaminated per-m mins at ~0.39 ns/elem
        instead of 0.52 (tensor_tensor 2x).
      * fine m-hi half + coarse: fp16 2x tensor_tensor running-min into acc
        tiles, PE-transpose partition collapse at the end (as v1).
  - HW-verified constraints: gpsimd/Pool tensor ops and DMA accum-min are
    ISA-rejected on TRN2; matmul psum out must be fp32; DVE 4x requires
    all-SBUF operands (which the transposed-S path arranges).

Host does only O(N) prep and the final scalar arithmetic.
"""

import sys

sys.path.insert(0, "/opt/trn_rl_repo")

import numpy as np

B = 8
NF = 8192  # fine points
NC_ = 1024  # coarse points
M = 8192  # gt points
MH = M // 2  # m-half size
NJ = MH // 128  # 32 j-blocks per half

# --- tuning knobs ----------------------------------------------------------
GT = 4  # X-route group size (tiles per transposed group)
CAST_DVE_MOD = 10  # every CAST_DVE_MOD-th cast group goes to VectorE

# --- module-level program cache -------------------------------------------
_PROGRAM = None
PROFILE = False
LAST_RESULTS = None


def _build_program():
    from concourse import bacc, bass, tile
    import concourse.mybir as mybir

    f16, f32 = mybir.dt.float16, mybir.dt.float32
    AL = mybir.AluOpType
    ACTF = mybir.ActivationFunctionType

    nc = bacc.Bacc("TRN2", target_bir_lowering=False, debug=False, num_devices=B)

    xaug_f = nc.dram_tensor("xaug_f", [7, NF], f16, kind="ExternalInput")
    xaug_c = nc.dram_tensor("xaug_c", [7, NC_], f16, kind="ExternalInput")
    yaug_d = nc.dram_tensor("yaug", [7, M], f16, kind="ExternalInput")
    iden_d = nc.dram_tensor("iden", [128, 128], f16, kind="ExternalInput")
    ones_d = nc.dram_tensor("ones128", [128, 1], f32, kind="ExternalInput")
    out_d = nc.dram_tensor("out", [1, 8], f32, kind="ExternalOutput")

    n_fine_tiles = NF // 128  # 64
    n_coarse_tiles = NC_ // 128  # 8
    assert n_fine_tiles % GT == 0

    cast_counter = [0]

    with tile.TileContext(nc) as tc:
        with (
            tc.tile_pool(name="const", bufs=1) as cpool,
            tc.tile_pool(name="s", bufs=4) as spool,
            tc.tile_pool(name="scr", bufs=2) as scrpool,
            tc.tile_pool(name="tg", bufs=2) as tgpool,
            tc.tile_pool(name="ct", bufs=2) as ctpool,
            tc.tile_pool(name="fin", bufs=1) as fpool,
            tc.tile_pool(name="ps", bufs=2, space=bass.MemorySpace.PSUM) as pspool,
        ):
            Xf = cpool.tile([7, NF], f16)
            nc.sync.dma_start(Xf[:], xaug_f.ap())
            Xc = cpool.tile([7, NC_], f16)
            nc.sync.dma_start(Xc[:], xaug_c.ap())
            Y = cpool.tile([7, M], f16)
            nc.sync.dma_start(Y[:], yaug_d.ap())
            iden = cpool.tile([128, 128], f16)
            nc.sync.dma_start(iden[:], iden_d.ap())
            ones = cpool.tile([128, 1], f32)
            nc.sync.dma_start(ones[:], ones_d.ap())

            outb = cpool.tile([1, 8], f32)

            accf = cpool.tile([128, MH], f16)  # fine m-hi col accumulator
            accc0 = cpool.tile([128, MH], f16)  # coarse m-lo
            accc1 = cpool.tile([128, MH], f16)  # coarse m-hi
            rowWf = cpool.tile([128, n_fine_tiles, 2], f32)
            rowWc = cpool.tile([128, n_coarse_tiles, 2], f32)
            colWX = cpool.tile([128, NJ], f32)  # X-route per-m col mins (m-lo)

            def make_S(Xa, i, mh):
                """matmuls + cast -> S fp16 [128, MH] (complete d), m-half mh."""
                S = spool.tile([128, MH], f16, tag="S")
                for g in range(2):
                    ps = pspool.tile([128, 2048], f32, tag="ps")
                    for j in range(4):
                        mlo = mh * MH + g * 2048 + j * 512
                        nc.tensor.matmul(
                            ps[:, j * 512 : (j + 1) * 512],
                            lhsT=Xa[:, i * 128 : (i + 1) * 128],
                            rhs=Y[:, mlo : mlo + 512],
                            start=True,
                            stop=True,
                        )
                    c = cast_counter[0]
                    cast_counter[0] += 1
                    if c % CAST_DVE_MOD == CAST_DVE_MOD - 1:
                        nc.vector.tensor_copy(S[:, g * 2048 : (g + 1) * 2048], ps[:])
                    else:
                        nc.scalar.activation(
                            S[:, g * 2048 : (g + 1) * 2048],
                            ps[:],
                            ACTF.Copy,
                            bias=0.0,
                            scale=1.0,
                        )
                return S

            def row_min(S, rowW, i, mh):
                scr = scrpool.tile([128, MH], f16, tag="scr")
                nc.vector.tensor_scalar(
                    out=scr[:],
                    in0=S[:],
                    scalar1=60000.0,
                    scalar2=None,
                    op0=AL.min,
                    op1=AL.min,
                    accum_out=rowW[:, i, mh : mh + 1],
                )

            # ---- fine family ----
            first_group = [True]

            def flush_group(TG):
                if first_group[0]:
                    tgt = colWX
                else:
                    tgt = ctpool.tile([128, NJ], f32, tag="colT")
                for j in range(NJ):
                    scr2 = scrpool.tile([128, GT * 128], f16, tag="scrj")
                    nc.vector.tensor_scalar(
                        out=scr2[:],
                        in0=TG[:, j, :, :],
                        scalar1=60000.0,
                        scalar2=None,
                        op0=AL.min,
                        op1=AL.min,
                        accum_out=tgt[:, j : j + 1],
                    )
                if tgt is not colWX:
                    nc.vector.tensor_tensor(
                        out=colWX[:], in0=colWX[:], in1=tgt[:], op=AL.min
                    )
                first_group[0] = False

            TG = None
            in_group = 0
            for i in range(n_fine_tiles):
                # m-lo half: X route
                S = make_S(Xf, i, 0)
                row_min(S, rowWf, i, 0)
                if in_group == 0:
                    TG = tgpool.tile([128, NJ, GT, 128], f16, tag="TG")
                nc.sync.dma_start(TG[:, :, in_group, :], S[:], transpose=True)
                in_group += 1
                if in_group == GT:
                    flush_group(TG)
                    in_group = 0
                # m-hi half: tt route
                S = make_S(Xf, i, 1)
                row_min(S, rowWf, i, 1)
                if i == 0:
                    nc.vector.tensor_copy(accf[:], S[:])
                else:
                    nc.vector.tensor_tensor(
                        out=accf[:], in0=accf[:], in1=S[:], op=AL.min
                    )
            assert in_group == 0

            # ---- coarse family: tt route both halves ----
            for i in range(n_coarse_tiles):
                for mh, acc in ((0, accc0), (1, accc1)):
                    S = make_S(Xc, i, mh)
                    row_min(S, rowWc, i, mh)
                    if i == 0:
                        nc.vector.tensor_copy(acc[:], S[:])
                    else:
                        nc.vector.tensor_tensor(
                            out=acc[:], in0=acc[:], in1=S[:], op=AL.min
                        )

            # ---- finals ----
            def row_total(rowW, nT, oidx):
                rmin = fpool.tile([128, nT], f32, tag=f"rmin{oidx}")
                nc.vector.tensor_reduce(
                    out=rmin[:], in_=rowW[:], axis=mybir.AxisListType.X, op=AL.min
                )
                rsum = fpool.tile([128, 1], f32, tag=f"rsum{oidx}")
                nc.vector.tensor_reduce(
                    out=rsum[:], in_=rmin[:], axis=mybir.AxisListType.X, op=AL.add
                )
                pr = pspool.tile([1, 1], f32, tag="ps")
                nc.tensor.matmul(pr[:], lhsT=rsum[:], rhs=ones[:], start=True, stop=True)
                nc.vector.tensor_copy(outb[0:1, oidx : oidx + 1], pr[:])

            def p_collapse(acc, cmb):
                """partition-collapse acc [128, MH] f16 -> cmb [128, NJ] f32."""
                for c0 in range(0, NJ, 4):
                    pst = pspool.tile([128, 4, 128], f16, tag="ps")
                    for q in range(4):
                        nc.tensor.transpose(
                            pst[:, q, :],
                            acc[:, (c0 + q) * 128 : (c0 + q + 1) * 128],
                            iden[:],
                        )
                    nc.vector.tensor_reduce(
                        out=cmb[:, c0 : c0 + 4],
                        in_=pst[:],
                        axis=mybir.AxisListType.X,
                        op=AL.min,
                    )

            def col_total(cmb, oidx):
                csum = fpool.tile([128, 1], f32, tag=f"csum{oidx}")
                nc.vector.tensor_reduce(
                    out=csum[:], in_=cmb[:], axis=mybir.AxisListType.X, op=AL.add
                )
                pc = pspool.tile([1, 1], f32, tag="ps")
                nc.tensor.matmul(pc[:], lhsT=csum[:], rhs=ones[:], start=True, stop=True)
                nc.vector.tensor_copy(outb[0:1, oidx : oidx + 1], pc[:])

            row_total(rowWf, n_fine_tiles, 0)
            row_total(rowWc, n_coarse_tiles, 2)

            # fine col: m-lo from colWX, m-hi from accf collapse; concat then sum
            colF = fpool.tile([128, 2 * NJ], f32, tag="colF")
            nc.vector.tensor_copy(colF[:, 0:NJ], colWX[:])
            p_collapse(accf, colF[:, NJ : 2 * NJ])
            col_total(colF, 1)

            colC = fpool.tile([128, 2 * NJ], f32, tag="colC")
            p_collapse(accc0, colC[:, 0:NJ])
            p_collapse(accc1, colC[:, NJ : 2 * NJ])
            col_total(colC, 3)

            nc.vector.memset(outb[0:1, 4:8], 0.0)
            nc.sync.dma_start(out_d.ap(), outb[:])

    nc.compile()
    return nc


def _get_program():
    global _PROGRAM
    if _PROGRAM is None:
        _PROGRAM = _build_program()
    return _PROGRAM


def _aug_x(x_b, n):
    """xaug [7, n]: rows x0,x1,x2, x2hi, x2lo, 1, 1 (fp16)."""
    f16 = np.float16
    xa = np.ones((7, n), f16)
    x16 = x_b.astype(f16)
    xa[0:3] = x16.T
    x2 = (x16.astype(np.float32) ** 2).sum(1)
    hi = x2.astype(f16)
    xa[3] = hi
    xa[4] = (x2 - hi.astype(np.float32)).astype(f16)
    return xa


def _prep_core_inputs(fine_b, coarse_b, gt_b):
    f16 = np.float16
    g16 = gt_b.astype(f16)  # [3, M]
    yaug = np.ones((7, M), f16)
    yaug[0:3] = (-2.0 * g16.astype(np.float32)).astype(f16)
    y2 = (g16.astype(np.float32) ** 2).sum(0)
    hi = y2.astype(f16)
    yaug[5] = hi
    yaug[6] = (y2 - hi.astype(np.float32)).astype(f16)
    return {
        "xaug_f": _aug_x(fine_b, NF),
        "xaug_c": _aug_x(coarse_b, NC_),
        "yaug": yaug,
        "iden": np.eye(128, dtype=f16),
        "ones128": np.ones((128, 1), np.float32),
    }


def kernel(coarse, fine, gt, alpha):
    global LAST_RESULTS
    from concourse import bass_utils

    coarse = np.asarray(coarse, np.float32)
    fine = np.asarray(fine, np.float32)
    gt = np.asarray(gt, np.float32)
    alpha = np.float32(np.asarray(alpha))

    nc = _get_program()
    in_maps = [_prep_core_inputs(fine[b], coarse[b], gt[b]) for b in range(B)]
    res = bass_utils.run_bass_kernel_spmd(
        nc, in_maps, core_ids=list(range(B)), trace=PROFILE
    )
    LAST_RESULTS = res
    per = np.stack([r["out"][0] for r in res.results]).astype(np.float64)  # [B, 8]
    lf = np.float32((per[:, 0] / NF + per[:, 1] / M).mean())
    lc = np.float32((per[:, 2] / NC_ + per[:, 3] / M).mean())
    loss = np.float32(lc + np.float32(alpha) * lf)
    return (loss, lc, lf)


if __name__ == "__main__":
    rng = np.random.default_rng(0)
    out = kernel(
        coarse=rng.standard_normal((B, NC_, 3)).astype(np.float32),
        fine=rng.standard_normal((B, NF, 3)).astype(np.float32),
        gt=rng.standard_normal((B, 3, M)).astype(np.float32),
        alpha=np.float32(1.0),
    )
    print(out)
